# revision 1
# baseline (speedup 1.0000x reference)
"""BERT-embedding kernel for Trainium2 (8 NeuronCores, data-parallel).

Computes, for input_sequence [256,512,10], doy_sequence [256,512] (int32),
W [256,10], b [256]:

    obs = input_sequence @ W.T + b          # [256,512,256]
    pos = PE_TABLE[doy_sequence]            # [256,512,256]
    out = concat([obs, pos], axis=-1)       # [256,512,512] fp32

Strategy: shard the batch dim 8 ways (32 batches / 16384 tokens per core),
replicate W/b and the 367x256 sinusoidal PE table. Per core the Bass kernel
pipelines, per 1024-token chunk:
  - gpsimd.dma_gather: PE rows (1KB each) HBM->SBUF keyed by the day-of-year
    index (tokens land on partition t%128, column t//128),
  - PE matmul producing the obs part in the same token layout. The fp32
    Linear is done as one K=33 fp16 matmul: x and W are split hi/lo in fp16
    and stacked as [x_hi; x_lo; x_hi] . [w_hi; w_hi; w_lo], which keeps
    fp32-grade accuracy (~3e-6 absmax) at 1 cycle/row (4x faster than the
    PE's native 2-pass fp32 mode). The bias is folded in via a ones-row.
  - two big HWDGE DMAs writing the obs / pos halves of the output rows.

Perf notes (from NTFF traces): the kernel is memory/Q7-bound - ~51 MB of
HBM traffic per core and 16384 SWDGE gather descriptors (~8.4 ns each on
the Q7 ucode). dynamic_dma_scratch_size is raised so the SWDGE ring never
wraps (reclaim-scan cost otherwise grows +0.5us per gather), and the mlp
ucode library is loaded explicitly up front so its ~10us IRAM fetch
overlaps the input loads instead of stalling the first gather.
"""

import math

import numpy as np

import concourse.bacc as bacc
import concourse.mybir as mybir
import concourse.tile as tile
from concourse.bass_utils import run_bass_kernel_spmd
from concourse.library_config import mlp

F32 = mybir.dt.float32
F16 = mybir.dt.float16
I16 = mybir.dt.int16

# Problem shapes (hardcoded per the harness contract).
B, S, NF = 256, 512, 10
E = 256
MAX_LEN = 366
N_CORES = 8
TOK = (B // N_CORES) * S          # tokens per core = 16384
CH = 1024                          # tokens per gather (HW limit ~1024 idxs)
NCH = TOK // CH                    # 16
COLS = CH // 128                   # 8
NROWS = 368                        # PE table rows padded (367 used)
KS = 33                            # stacked fp16 hi/lo contraction dim

_COMPILED_NC = None
_LAST_RESULTS = None               # BassKernelResults of the most recent run


def _make_pe() -> np.ndarray:
    """Sinusoidal table, row 0 zeros (padding), rows 1..366 = positions 0..365."""
    pe = np.zeros((NROWS, E), dtype=np.float32)
    position = np.arange(0, MAX_LEN, dtype=np.float32)[:, None]
    div_term = np.exp(
        np.arange(0, E, 2, dtype=np.float32) * -(math.log(10000.0) / E)
    )
    pe[1 : MAX_LEN + 1, 0::2] = np.sin(position * div_term)
    pe[1 : MAX_LEN + 1, 1::2] = np.cos(position * div_term)
    return pe


def _build():
    nc = bacc.Bacc(
        "TRN2",
        target_bir_lowering=False,
        debug=False,
        dynamic_dma_scratch_size=32768,
        num_swdge_queues=2,
    )
    xT = nc.dram_tensor("xT", [KS, TOK], F16, kind="ExternalInput")
    wT = nc.dram_tensor("wT", [KS, E], F16, kind="ExternalInput")
    pe = nc.dram_tensor("pe", [NROWS, E], F32, kind="ExternalInput")
    idx = nc.dram_tensor("idx", [128, TOK // 16], I16, kind="ExternalInput")
    out = nc.dram_tensor("out", [TOK, 2 * E], F32, kind="ExternalOutput")

    # out viewed as [chunk, half, partition, col, 256]: token (cc*COLS+j)*128+p
    out5 = out.ap().rearrange("(cc j p) (h e) -> cc h p j e", p=128, j=COLS, h=2)

    with tile.TileContext(nc) as tc:
        with (
            tc.tile_pool(name="const", bufs=1) as const_pool,
            tc.tile_pool(name="pos", bufs=8) as pos_pool,
            tc.tile_pool(name="obs", bufs=4) as obs_pool,
            tc.tile_pool(name="psum", bufs=8, space="PSUM") as psum_pool,
        ):
            # Load the Q7 gather ucode immediately; its IRAM DMA overlaps
            # the input loads below (all on HWDGE rings, not gpsimd).
            nc.gpsimd.load_library(mlp)
            idx_sb = const_pool.tile([128, TOK // 16], I16, tag="idx_sb")
            nc.sync.dma_start(out=idx_sb[:], in_=idx[:, :])
            wT_sb = const_pool.tile([KS, E], F16, tag="wT_sb")
            nc.scalar.dma_start(out=wT_sb[:], in_=wT[:, :])
            xT_sb = const_pool.tile([KS, TOK], F16, tag="xT_sb")
            # 4 chunked loads: [33, TOK] uses only 33 partitions (~95 GB/s),
            # so chunking lets early matmuls start before the full load lands.
            for q4 in range(4):
                nc.scalar.dma_start(
                    out=xT_sb[:, q4 * (TOK // 4) : (q4 + 1) * (TOK // 4)],
                    in_=xT[:, q4 * (TOK // 4) : (q4 + 1) * (TOK // 4)],
                )

            for c in range(NCH):
                pos_t = pos_pool.tile([128, COLS, E], F32, tag="pos_t")
                # Alternating SWDGE queues: queue-1 gathers pipeline behind
                # queue-0 on the Q7, doubling effective desc-gen throughput.
                nc.gpsimd.dma_gather(
                    pos_t[:],
                    pe[:, :],
                    idx_sb[:, c * (CH // 16) : (c + 1) * (CH // 16)],
                    CH,
                    CH,
                    E,
                    queue_num=c % 2,
                )
                nc.sync.dma_start(out=out5[c, 1], in_=pos_t[:])

                obs_t = obs_pool.tile([128, COLS, E], F32, tag="obs_t")
                for k in range(COLS):
                    ps = psum_pool.tile([128, E], F32, tag="ps")
                    t0 = (c * COLS + k) * 128
                    nc.tensor.matmul(
                        out=ps[:],
                        lhsT=xT_sb[:, t0 : t0 + 128],
                        rhs=wT_sb[:],
                        start=True,
                        stop=True,
                    )
                    nc.vector.tensor_copy(out=obs_t[:, k, :], in_=ps[:])
                nc.scalar.dma_start(out=out5[c, 0], in_=obs_t[:])
    nc.compile()
    return nc


def kernel(input_sequence, doy_sequence, W, b) -> np.ndarray:
    global _COMPILED_NC, _LAST_RESULTS

    x = np.ascontiguousarray(np.asarray(input_sequence, dtype=np.float32))
    doy = np.asarray(doy_sequence, dtype=np.int32)
    W = np.asarray(W, dtype=np.float32)
    bias = np.asarray(b, dtype=np.float32)

    if _COMPILED_NC is None:
        _COMPILED_NC = _build()
    nc = _COMPILED_NC

    # Augmented weights [11, E]: rows 0..9 = W.T, row 10 = bias (ones-row
    # trick); then fp16 hi/lo stacking [w_hi; w_hi; w_lo] -> [33, E].
    wTf = np.concatenate([W.T, bias[None, :]], axis=0).astype(np.float32)
    wh = wTf.astype(np.float16)
    wl = (wTf - wh.astype(np.float32)).astype(np.float16)
    wT = np.ascontiguousarray(np.concatenate([wh, wh, wl], axis=0))
    petab = _make_pe()

    bpc = B // N_CORES
    in_maps = []
    for c in range(N_CORES):
        xc = x[c * bpc : (c + 1) * bpc].reshape(TOK, NF)
        xTf = np.empty((NF + 1, TOK), dtype=np.float32)
        xTf[:NF] = xc.T
        xTf[NF] = 1.0
        xh = xTf.astype(np.float16)
        xl = (xTf - xh.astype(np.float32)).astype(np.float16)
        xT = np.ascontiguousarray(np.concatenate([xh, xl, xh], axis=0))
        ids = doy[c * bpc : (c + 1) * bpc].reshape(TOK).astype(np.int16)
        idx_wrapped = np.tile(np.ascontiguousarray(ids.reshape(-1, 16).T), (8, 1))
        in_maps.append({"xT": xT, "wT": wT, "pe": petab, "idx": idx_wrapped})

    _LAST_RESULTS = run_bass_kernel_spmd(nc, in_maps, core_ids=list(range(N_CORES)))

    out = np.empty((B, S, 2 * E), dtype=np.float32)
    for c in range(N_CORES):
        out[c * bpc : (c + 1) * bpc] = _LAST_RESULTS.results[c]["out"].reshape(
            bpc, S, 2 * E
        )
    return out



# revision 3
# speedup vs baseline: 1.3181x; 1.3181x over previous
"""BERT-embedding kernel for Trainium2 (8 NeuronCores, data-parallel).

Computes, for input_sequence [256,512,10], doy_sequence [256,512] (int32),
W [256,10], b [256]:

    obs = input_sequence @ W.T + b          # [256,512,256]
    pos = PE_TABLE[doy_sequence]            # [256,512,256]
    out = concat([obs, pos], axis=-1)       # [256,512,512] fp32

Strategy: shard the batch dim 8 ways (32 batches / 16384 tokens per core),
replicate W/b and the 367x256 sinusoidal PE table. The kernel is HBM-write
bound (33.5 MB of output per core), so the whole point of the design is to
spend NO extra HBM traffic beyond the ~0.6 MB of inputs:

  - The PE table lives in SBUF as fp16 [128, 3*256] (row r = 128*k + p).
  - pos rows are produced by a one-hot matmul instead of a DMA gather:
    a K=1 ones-matmul broadcasts doy across all 128 partitions into PSUM,
    the vector engine compares it against per-partition row ids (is_equal)
    to build onehot^T [row, token] fp16, and three accumulated fp16 matmuls
    per 128-token group compute onehot^T.T @ pe (table rows 0..127,
    128..255, 256..383).
  - obs is one K=11 fp16 matmul ([x^T; ones] . [W^T; b]) into the same
    PSUM bank, columns 0:256 (plain fp16 is ~1e-3 absolute error, far
    inside the 2e-2 gate; fp16 halves the x upload vs fp32).
  - One PSUM->SBUF copy per group (vector/scalar engines alternating) and
    one token-major 1 MB HWDGE DMA per 512-token chunk writes rows as
    contiguous 2 KB descriptors at full DMA-bus efficiency.

vs. the previous SWDGE-gather version this removes 16.8 MB of random 1KB
HBM reads and all Q7 descriptor generation; per-core DMA drops to
~34 MB ~= the output-write roofline.
"""

import math

import numpy as np

import concourse.bacc as bacc
import concourse.mybir as mybir
import concourse.tile as tile
from concourse.bass_utils import run_bass_kernel_spmd

F32 = mybir.dt.float32
F16 = mybir.dt.float16

# Problem shapes (hardcoded per the harness contract).
B, S, NF = 256, 512, 10
E = 256
MAX_LEN = 366
N_CORES = 8
TOK = (B // N_CORES) * S          # tokens per core = 16384
CH = 512                           # tokens per chunk
GRP = CH // 128                    # 128-token groups per chunk = 4
NCH = TOK // CH                    # 32
KF = NF + 1                        # obs contraction dim (features + bias row)
NKC = 3                            # one-hot row chunks (384 padded rows)

_COMPILED_NC = None
_LAST_RESULTS = None               # BassKernelResults of the most recent run


def _make_pe() -> np.ndarray:
    """Sinusoidal table, row 0 zeros (padding), rows 1..366 = positions 0..365."""
    pe = np.zeros((128 * NKC, E), dtype=np.float32)
    position = np.arange(0, MAX_LEN, dtype=np.float32)[:, None]
    div_term = np.exp(
        np.arange(0, E, 2, dtype=np.float32) * -(math.log(10000.0) / E)
    )
    pe[1 : MAX_LEN + 1, 0::2] = np.sin(position * div_term)
    pe[1 : MAX_LEN + 1, 1::2] = np.cos(position * div_term)
    return pe


def _build():
    nc = bacc.Bacc("TRN2", target_bir_lowering=False, debug=False)
    xT = nc.dram_tensor("xT", [KF, TOK], F16, kind="ExternalInput")
    wT = nc.dram_tensor("wT", [KF, E], F16, kind="ExternalInput")
    pe3 = nc.dram_tensor("pe3", [128, NKC * E], F16, kind="ExternalInput")
    doy = nc.dram_tensor("doy", [1, TOK], F16, kind="ExternalInput")
    cmp = nc.dram_tensor("cmp", [128, NKC], F32, kind="ExternalInput")
    out = nc.dram_tensor("out", [TOK, 2 * E], F32, kind="ExternalOutput")

    # out viewed as [chunk, partition, group, 512]: token (c*GRP+j)*128+p
    out4 = out.ap().rearrange("(c j p) e -> c p j e", p=128, j=GRP)

    with tile.TileContext(nc) as tc:
        with (
            tc.tile_pool(name="const", bufs=1) as const_pool,
            tc.tile_pool(name="oh", bufs=3) as oh_pool,
            tc.tile_pool(name="ot", bufs=3) as ot_pool,
            tc.tile_pool(name="psA", bufs=2, space="PSUM") as psA,
            tc.tile_pool(name="psB", bufs=6, space="PSUM") as psB,
        ):
            wT_sb = const_pool.tile([KF, E], F16, tag="wT_sb")
            nc.scalar.dma_start(out=wT_sb[:], in_=wT[:, :])
            pe_sb = const_pool.tile([128, NKC * E], F16, tag="pe_sb")
            nc.scalar.dma_start(out=pe_sb[:], in_=pe3[:, :])
            cmp_sb = const_pool.tile([128, NKC], F32, tag="cmp_sb")
            nc.scalar.dma_start(out=cmp_sb[:], in_=cmp[:, :])
            doy_sb = const_pool.tile([1, TOK], F16, tag="doy_sb")
            nc.sync.dma_start(out=doy_sb[:], in_=doy[:, :])
            ones_sb = const_pool.tile([1, 128], F16, tag="ones_sb")
            nc.vector.memset(ones_sb[:], 1.0)
            xT_sb = const_pool.tile([KF, TOK], F16, tag="xT_sb")
            # Chunked loads so early matmuls start before the full load lands.
            for q in range(4):
                nc.sync.dma_start(
                    out=xT_sb[:, q * (TOK // 4) : (q + 1) * (TOK // 4)],
                    in_=xT[:, q * (TOK // 4) : (q + 1) * (TOK // 4)],
                )

            for c in range(NCH):
                # doyb[p, t] = doy[t] for the chunk's 512 tokens (K=1 matmul
                # broadcast; exact for integer-valued fp16 inputs).
                dbp = psA.tile([128, CH], F32, tag="dbp")
                nc.tensor.matmul(
                    out=dbp[:],
                    lhsT=ones_sb[:],
                    rhs=doy_sb[:, c * CH : (c + 1) * CH],
                    start=True,
                    stop=True,
                )
                # onehot^T[p, k, t] = (doy[t] == 128*k + p)
                oh = oh_pool.tile([128, NKC, CH], F16, tag="oh")
                for k in range(NKC):
                    nc.vector.tensor_scalar(
                        out=oh[:, k, :],
                        in0=dbp[:],
                        scalar1=cmp_sb[:, k : k + 1],
                        scalar2=None,
                        op0=mybir.AluOpType.is_equal,
                    )

                ot = ot_pool.tile([128, GRP, 2 * E], F32, tag="ot")
                for j in range(GRP):
                    ps = psB.tile([128, 2 * E], F32, tag="ps")
                    t0 = c * CH + j * 128
                    nc.tensor.matmul(
                        out=ps[:, 0:E],
                        lhsT=xT_sb[:, t0 : t0 + 128],
                        rhs=wT_sb[:],
                        start=True,
                        stop=True,
                    )
                    for k in range(NKC):
                        nc.tensor.matmul(
                            out=ps[:, E : 2 * E],
                            lhsT=oh[:, k, j * 128 : (j + 1) * 128],
                            rhs=pe_sb[:, k * E : (k + 1) * E],
                            start=(k == 0),
                            stop=(k == NKC - 1),
                        )
                    # Alternate PSUM->SBUF copies across vector/scalar engines.
                    if j < 2:
                        nc.vector.tensor_copy(out=ot[:, j, :], in_=ps[:])
                    else:
                        nc.scalar.activation(
                            out=ot[:, j, :],
                            in_=ps[:],
                            func=mybir.ActivationFunctionType.Copy,
                        )
                eng = nc.sync if c % 2 == 0 else nc.scalar
                eng.dma_start(out=out4[c], in_=ot[:])
    nc.compile()
    return nc


def kernel(input_sequence, doy_sequence, W, b) -> np.ndarray:
    global _COMPILED_NC, _LAST_RESULTS

    x = np.asarray(input_sequence, dtype=np.float32)
    doy = np.asarray(doy_sequence, dtype=np.int32)
    W = np.asarray(W, dtype=np.float32)
    bias = np.asarray(b, dtype=np.float32)

    if _COMPILED_NC is None:
        _COMPILED_NC = _build()
    nc = _COMPILED_NC

    # Augmented weights [11, E]: rows 0..9 = W.T, row 10 = bias (ones-row).
    wTf = np.concatenate([W.T, bias[None, :]], axis=0)
    wT = np.ascontiguousarray(wTf.astype(np.float16))

    # PE table fp16, packed [128, 3*256]: row r=128k+p at pe3[p, k*256:...].
    petab = _make_pe().astype(np.float16)
    pe3 = np.ascontiguousarray(
        petab.reshape(NKC, 128, E).transpose(1, 0, 2).reshape(128, NKC * E)
    )

    # Per-partition compare constants: cmp[p, k] = p + 128k.
    cmpc = np.ascontiguousarray(
        (np.arange(128, dtype=np.float32)[:, None]
         + 128.0 * np.arange(NKC, dtype=np.float32)[None, :])
    )

    bpc = B // N_CORES
    in_maps = []
    for c in range(N_CORES):
        xc = x[c * bpc : (c + 1) * bpc].reshape(TOK, NF)
        xTf = np.empty((KF, TOK), dtype=np.float16)
        xTf[:NF] = xc.T.astype(np.float16)
        xTf[NF] = 1.0
        doy_c = np.ascontiguousarray(
            doy[c * bpc : (c + 1) * bpc].reshape(1, TOK).astype(np.float16)
        )
        in_maps.append(
            {"xT": np.ascontiguousarray(xTf), "wT": wT, "pe3": pe3,
             "doy": doy_c, "cmp": cmpc}
        )

    _LAST_RESULTS = run_bass_kernel_spmd(nc, in_maps, core_ids=list(range(N_CORES)))

    out = np.empty((B, S, 2 * E), dtype=np.float32)
    for c in range(N_CORES):
        out[c * bpc : (c + 1) * bpc] = _LAST_RESULTS.results[c]["out"].reshape(
            bpc, S, 2 * E
        )
    return out
